# revision 1
# baseline (speedup 1.0000x reference)
"""DGCNConv (GNN message passing) Trainium2 kernel, 8-core SPMD.

Strategy (graph/data parallel, per sharding hint):
- Nodes are partitioned into 8 contiguous ranges of 6250. Core c owns the
  dst-range edges for agg_in and the src-range edges for agg_out.
- Per direction, edges are sorted by (target block of 128, source half),
  gathered from HBM with the SWDGE dma_gather (256B rows, fp16 duplicated
  x table), and segment-reduced on-chip with one-hot matmuls accumulating
  in PSUM (aggT layout [feat, node]).
- Epilogue: outT = W_self@xT + W_in@agg_inT + W_out@agg_outT per 128-node
  block, ReLU (with running sums for BN), cross-core AllReduce of BN
  partials, then fused scale/shift normalize and writeback.
- Host: routes/pads edges, builds index tables, transposes the output back.

The per-(block,half) segment sizes are padded to the max over cores so all
8 cores run one identical program (SPMD NEFF) on per-core data.
"""

import sys

if "/opt/trn_rl_repo" not in sys.path:
    sys.path.insert(0, "/opt/trn_rl_repo")

import numpy as np

N_NODES = 50000
N_EDGES = 800000
D = 64
N_CORES = 8
NPC = N_NODES // N_CORES          # 6250 nodes per core
NBLK = (NPC + 127) // 128         # 49 blocks per core
HALF = N_NODES // 2               # 25000, int16-safe gather base split
BN_EPS = 1e-5
GCAP = 1024                       # max positions per dma_gather instruction (SWDGE ring holds 1024 descs)
MB = 16                           # 128-edge chunks per one-hot build batch


# ---------------------------------------------------------------- host prep

def _route_direction(t_all, g_all):
    """Route edges (t = reduce-target node id, g = gather node id) to cores.

    Returns (layout, per_core) where layout describes the static position
    map shared by all cores and per_core holds each core's idx/dcmp arrays.
    """
    core_of = t_all // NPC
    per_core_edges = []
    for c in range(N_CORES):
        m = core_of == c
        t = t_all[m] - c * NPC
        g = g_all[m]
        blk = t >> 7
        half = (g >= HALF).astype(np.int64)
        halfkey = np.where((blk & 1) == 0, half, 1 - half)
        order = np.lexsort((g, halfkey, blk))
        t, g, blk, half = t[order], g[order], blk[order], half[order]
        # counts per (blk, half) in layout order
        per_core_edges.append((t, g, blk, half))

    # static budgets per (blk, halfslot) where halfslot 0/1 is layout order
    budgets = np.zeros((NBLK, 2), np.int64)
    for c in range(N_CORES):
        t, g, blk, half = per_core_edges[c]
        halfslot = np.where((blk & 1) == 0, half, 1 - half)
        cnt = np.bincount(blk * 2 + halfslot, minlength=NBLK * 2).reshape(NBLK, 2)
        budgets = np.maximum(budgets, cnt)
    budgets = ((budgets + 127) // 128) * 128  # pad to whole chunks

    # layout: position ranges per (blk, halfslot)
    seg_start = np.zeros((NBLK, 2), np.int64)
    pos = 0
    for b in range(NBLK):
        for hs in range(2):
            seg_start[b, hs] = pos
            pos += budgets[b, hs]
    total = pos
    nch = total // 128

    # gather runs: merge consecutive same-half segments, cap at GCAP
    runs = []  # (pos0, npos, half)
    for b in range(NBLK):
        for hs in range(2):
            if budgets[b, hs] == 0:
                continue
            h = hs if (b & 1) == 0 else 1 - hs
            p0, n = int(seg_start[b, hs]), int(budgets[b, hs])
            if runs and runs[-1][2] == h and runs[-1][0] + runs[-1][1] == p0 \
                    and runs[-1][1] + n <= GCAP:
                runs[-1] = (runs[-1][0], runs[-1][1] + n, h)
            else:
                while n > GCAP:
                    runs.append((p0, GCAP, h))
                    p0 += GCAP
                    n -= GCAP
                runs.append((p0, n, h))

    # block chunk ranges
    blocks = []  # (blk, c0, c1)
    for b in range(NBLK):
        c0 = int(seg_start[b, 0]) // 128
        c1 = (int(seg_start[b, 1]) + int(budgets[b, 1])) // 128
        blocks.append((b, c0, c1))

    # chunk -> run mapping
    chunk_run = np.zeros(nch, np.int64)
    run_c0 = np.zeros(len(runs), np.int64)
    for ri, (p0, n, h) in enumerate(runs):
        run_c0[ri] = p0 // 128
        chunk_run[p0 // 128:(p0 + n) // 128] = ri

    # per-core position arrays
    per_core = []
    for c in range(N_CORES):
        t, g, blk, half = per_core_edges[c]
        idx = np.zeros(total, np.int16)          # gather idx rel to half base
        dcmp = np.full(total, -1.0, np.float16)  # one-hot compare value
        halfslot = np.where((blk & 1) == 0, half, 1 - half)
        key = blk * 2 + halfslot
        cnt = np.bincount(key, minlength=NBLK * 2)
        # edges are sorted by (blk, halfslot) so each segment is contiguous
        estart = np.zeros(NBLK * 2, np.int64)
        estart[1:] = np.cumsum(cnt)[:-1]
        for b in range(NBLK):
            for hs in range(2):
                n = int(cnt[b * 2 + hs])
                if n == 0:
                    continue
                e0 = int(estart[b * 2 + hs])
                p0 = int(seg_start[b, hs])
                h = hs if (b & 1) == 0 else 1 - hs
                idx[p0:p0 + n] = (g[e0:e0 + n] - h * HALF).astype(np.int16)
                dcmp[p0:p0 + n] = (t[e0:e0 + n] - b * 128).astype(np.float16)
        idx_wrapped = np.tile(
            np.ascontiguousarray(idx.reshape(-1, 16).T), (8, 1))
        dcmp_wrapped = np.ascontiguousarray(dcmp.reshape(-1, 128).T)
        per_core.append((idx_wrapped, dcmp_wrapped))

    layout = dict(total=total, nch=nch, runs=runs, blocks=blocks,
                  chunk_run=chunk_run, run_c0=run_c0)
    return layout, per_core


# ---------------------------------------------------------------- program

def _build_program(lay_in, lay_out):
    import concourse.bacc as bacc
    import concourse.mybir as mybir
    from concourse import tile
    from concourse import library_config

    f32, f16, i16 = mybir.dt.float32, mybir.dt.float16, mybir.dt.int16
    nc = bacc.Bacc(None, target_bir_lowering=False, debug=False)

    xdup = nc.dram_tensor("xdup", [N_NODES, 2 * D], f16, kind="ExternalInput")
    xT_d = nc.dram_tensor("xT", [D, NPC], f32, kind="ExternalInput")
    Wt_d = nc.dram_tensor("Wt", [D, 3 * D], f32, kind="ExternalInput")
    gb_d = nc.dram_tensor("gb", [D, 2], f32, kind="ExternalInput")
    out_d = nc.dram_tensor("out", [D, NPC], f32, kind="ExternalOutput")
    cc_in = nc.dram_tensor("cc_in", [D, 2], f32)
    cc_out = nc.dram_tensor("cc_out", [D, 2], f32, addr_space="Shared")

    lays = {"in": lay_in, "out": lay_out}
    idx_d, dcmp_d = {}, {}
    for dk in ("in", "out"):
        tot = lays[dk]["total"]
        idx_d[dk] = nc.dram_tensor(
            f"idx_{dk}", [128, tot // 16], i16, kind="ExternalInput")
        dcmp_d[dk] = nc.dram_tensor(
            f"dcmp_{dk}", [128, tot // 128], f16, kind="ExternalInput")

    with tile.TileContext(nc) as tc:
        nc.gpsimd.load_library(library_config.mlp)
        with (
            tc.tile_pool(name="const", bufs=1) as cpool,
            tc.tile_pool(name="gath", bufs=3) as gpool,
            tc.tile_pool(name="mb", bufs=3) as mpool,
            tc.tile_pool(name="dr", bufs=2) as dpool,
            tc.tile_pool(name="agg_ps", bufs=3, space="PSUM") as agg_pspool,
            tc.tile_pool(name="out_ps", bufs=2, space="PSUM") as out_pspool,
        ):
            # --- constants
            xT = cpool.tile([D, NPC], f32, tag="xT")
            nc.sync.dma_start(xT[:], xT_d[:])
            Wt = cpool.tile([D, 3 * D], f32, tag="Wt")
            nc.sync.dma_start(Wt[:], Wt_d[:])
            gb = cpool.tile([D, 2], f32, tag="gb")
            nc.sync.dma_start(gb[:], gb_d[:])
            iota_i = cpool.tile([128, MB, 128], i16, tag="iota_i")
            nc.gpsimd.iota(iota_i[:], [[0, MB], [1, 128]], base=0,
                           channel_multiplier=0)
            iota_f = cpool.tile([128, MB, 128], f16, tag="iota_f")
            nc.vector.tensor_copy(iota_f[:], iota_i[:])

            idx_t, dcmp_t = {}, {}
            for dk in ("in", "out"):
                tot = lays[dk]["total"]
                idx_t[dk] = cpool.tile([128, tot // 16], i16, tag=f"idx{dk}", name=f"idx_t_{dk}")
                nc.sync.dma_start(idx_t[dk][:], idx_d[dk][:])
                dcmp_t[dk] = cpool.tile([128, tot // 128], f16, tag=f"dc{dk}", name=f"dcmp_t_{dk}")
                nc.sync.dma_start(dcmp_t[dk][:], dcmp_d[dk][:])

            agg_sb = {}
            for dk in ("in", "out"):
                agg_sb[dk] = cpool.tile([D, NBLK * 128], f32, tag=f"agg{dk}", name=f"agg_sb_{dk}")

            # --- per-direction segment reduce
            for dk in ("in", "out"):
                lay = lays[dk]
                runs, blocks = lay["runs"], lay["blocks"]
                chunk_run, run_c0 = lay["chunk_run"], lay["run_c0"]
                nch = lay["nch"]
                g_tiles, m_tiles = {}, {}

                def emit_run(ri, dk=dk, runs=runs, g_tiles=g_tiles):
                    p0, npos, h = runs[ri]
                    gt = gpool.tile([128, npos // 128, 2 * D], f16, tag="g", name=f"g_{dk}_{ri}")
                    src = xdup[h * HALF:(h + 1) * HALF, :]
                    nc.gpsimd.dma_gather(
                        gt[:], src, idx_t[dk][:, p0 // 16:(p0 + npos) // 16],
                        npos, npos, 2 * D)
                    g_tiles[ri] = gt

                def emit_batch(bi, dk=dk, nch=nch, m_tiles=m_tiles):
                    c0 = bi * MB
                    nb = min(MB, nch - c0)
                    dr = dpool.tile([128, nb, 128], f16, tag="drep", name=f"dr_{dk}_{bi}")
                    nc.vector.tensor_copy(
                        dr[:],
                        dcmp_t[dk][:, c0:c0 + nb].unsqueeze(2)
                        .broadcast_to([128, nb, 128]))
                    mt = mpool.tile([128, nb, 128], f16, tag="m", name=f"m_{dk}_{bi}")
                    nc.vector.tensor_tensor(
                        mt[:], iota_f[:, :nb, :], dr[:],
                        op=mybir.AluOpType.is_equal)
                    m_tiles[bi] = mt

                for b, c0, c1 in blocks:
                    if c1 == c0:
                        nc.vector.memset(
                            agg_sb[dk][:, b * 128:(b + 1) * 128], 0.0)
                        continue
                    aps = agg_pspool.tile([D, 128], f32, tag="aggps", name=f"aps_{dk}_{b}")
                    for c in range(c0, c1):
                        ri = int(chunk_run[c])
                        if ri not in g_tiles:
                            emit_run(ri)
                        bi = c // MB
                        if bi not in m_tiles:
                            emit_batch(bi)
                        nc.tensor.matmul(
                            aps[:],
                            g_tiles[ri][:, c - int(run_c0[ri]), 0:D],
                            m_tiles[bi][:, c - bi * MB, :],
                            start=(c == c0), stop=(c == c1 - 1))
                    nc.vector.tensor_copy(
                        agg_sb[dk][:, b * 128:(b + 1) * 128], aps[:])

            # --- epilogue: linear + relu + BN partial sums
            r_sb = cpool.tile([D, NPC], f32, tag="r")
            sums = cpool.tile([D, NBLK], f32, tag="sums")
            sumsq = cpool.tile([D, NBLK], f32, tag="sumsq")
            sq_scr = dpool.tile([D, 128], f32, tag="sq")
            for b in range(NBLK):
                ncols = min(128, NPC - b * 128)
                ops = out_pspool.tile([D, ncols], f32, tag="outps", name=f"ops_{b}")
                nc.tensor.matmul(ops[:], Wt[:, 0:D],
                                 xT[:, b * 128:b * 128 + ncols],
                                 start=True, stop=False)
                nc.tensor.matmul(ops[:], Wt[:, D:2 * D],
                                 agg_sb["in"][:, b * 128:b * 128 + ncols],
                                 start=False, stop=False)
                nc.tensor.matmul(ops[:], Wt[:, 2 * D:3 * D],
                                 agg_sb["out"][:, b * 128:b * 128 + ncols],
                                 start=False, stop=True)
                nc.scalar.activation(
                    r_sb[:, b * 128:b * 128 + ncols], ops[:],
                    mybir.ActivationFunctionType.Relu,
                    accum_out=sums[:, b:b + 1])
                nc.scalar.activation(
                    sq_scr[:, 0:ncols], r_sb[:, b * 128:b * 128 + ncols],
                    mybir.ActivationFunctionType.Square,
                    accum_out=sumsq[:, b:b + 1])

            # --- BN stats allreduce
            part = cpool.tile([D, 2], f32, tag="part")
            nc.vector.tensor_reduce(part[:, 0:1], sums[:],
                                    mybir.AxisListType.X, mybir.AluOpType.add)
            nc.vector.tensor_reduce(part[:, 1:2], sumsq[:],
                                    mybir.AxisListType.X, mybir.AluOpType.add)
            nc.sync.dma_start(cc_in[:], part[:])
            nc.gpsimd.collective_compute(
                "AllReduce", mybir.AluOpType.add,
                replica_groups=[list(range(N_CORES))],
                ins=[cc_in[:]], outs=[cc_out[:]])
            tot = cpool.tile([D, 2], f32, tag="tot")
            nc.sync.dma_start(tot[:], cc_out[:])

            # --- scale/shift
            stats = cpool.tile([D, 8], f32, tag="stats")
            mean, ex2 = stats[:, 0:1], stats[:, 1:2]
            var, std = stats[:, 2:3], stats[:, 3:4]
            inv, scale = stats[:, 4:5], stats[:, 5:6]
            shift, tmp = stats[:, 6:7], stats[:, 7:8]
            inv_n = 1.0 / float(N_NODES)
            nc.vector.tensor_scalar_mul(mean, tot[:, 0:1], inv_n)
            nc.vector.tensor_scalar_mul(ex2, tot[:, 1:2], inv_n)
            nc.vector.tensor_tensor(tmp, mean, mean, op=mybir.AluOpType.mult)
            nc.vector.tensor_tensor(var, ex2, tmp,
                                    op=mybir.AluOpType.subtract)
            nc.vector.tensor_scalar_add(var, var, BN_EPS)
            nc.scalar.activation(std, var, mybir.ActivationFunctionType.Sqrt)
            nc.vector.reciprocal(inv, std)
            nc.vector.tensor_tensor(scale, gb[:, 0:1], inv,
                                    op=mybir.AluOpType.mult)
            nc.vector.tensor_tensor(tmp, mean, scale,
                                    op=mybir.AluOpType.mult)
            nc.vector.tensor_tensor(shift, gb[:, 1:2], tmp,
                                    op=mybir.AluOpType.subtract)

            # --- normalize + writeback
            out_sb = cpool.tile([D, NPC], f32, tag="outsb")
            nc.vector.tensor_scalar(out_sb[:], r_sb[:], scale, shift,
                                    op0=mybir.AluOpType.mult,
                                    op1=mybir.AluOpType.add)
            nc.sync.dma_start(out_d[:], out_sb[:])

    nc.finalize()
    return nc


# ---------------------------------------------------------------- kernel

def prepare(x, edge_index, num_nodes=None, W_in=None, W_out=None,
            W_self=None, gamma=None, beta=None):
    """Build the bass program and per-core input maps."""
    x = np.asarray(x, np.float32)
    edge_index = np.asarray(edge_index, np.int64)
    W_in = np.asarray(W_in, np.float32)
    W_out = np.asarray(W_out, np.float32)
    W_self = np.asarray(W_self, np.float32)
    gamma = np.asarray(gamma, np.float32)
    beta = np.asarray(beta, np.float32)
    assert x.shape == (N_NODES, D) and edge_index.shape == (2, N_EDGES)

    src, dst = edge_index[0], edge_index[1]
    lay_in, pc_in = _route_direction(dst, src)    # agg_in: reduce by dst
    lay_out, pc_out = _route_direction(src, dst)  # agg_out: reduce by src

    nc = _build_program(lay_in, lay_out)

    x16 = x.astype(np.float16)
    xdup = np.concatenate([x16, x16], axis=1)
    Wt = np.concatenate([W_self.T, W_in.T, W_out.T], axis=1).astype(np.float32)
    Wt = np.ascontiguousarray(Wt)
    gb = np.ascontiguousarray(np.stack([gamma, beta], axis=1).astype(np.float32))

    in_maps = []
    for c in range(N_CORES):
        xT_c = np.ascontiguousarray(x[c * NPC:(c + 1) * NPC].T)
        in_maps.append({
            "xdup": xdup,
            "xT": xT_c,
            "Wt": Wt,
            "gb": gb,
            "idx_in": pc_in[c][0], "dcmp_in": pc_in[c][1],
            "idx_out": pc_out[c][0], "dcmp_out": pc_out[c][1],
        })
    return nc, in_maps


def postprocess(results):
    outT = np.concatenate([r["out"] for r in results], axis=1)
    return np.ascontiguousarray(outT.T).astype(np.float32)


def kernel(x, edge_index, num_nodes=None, W_in=None, W_out=None,
           W_self=None, gamma=None, beta=None):
    from concourse.bass_utils import run_bass_kernel_spmd

    nc, in_maps = prepare(x, edge_index, num_nodes, W_in, W_out,
                          W_self, gamma, beta)
    res = run_bass_kernel_spmd(nc, in_maps, core_ids=list(range(N_CORES)))
    return postprocess(res.results)

